# revision 1
# baseline (speedup 1.0000x reference)
"""Trainium2 Bass kernel for pre-norm causal attention block.

Module: out = x + Wo(attn(LN(x))) with fused QKV, 16 heads, causal mask.
Shapes (hardcoded): x [2, 2048, 1024], wqkv [1024, 3072], wo [1024, 1024].

Sharding (8 cores, one program SPMD):
  core c = 4*b + s  handles batch b, global heads [4s, 4s+4), and — after an
  in-group AllToAll of the attention context — output rows for seq chunk
  [512 s, 512 s + 512) of its batch.

Dataflow per core (everything feature-on-partitions, i.e. transposed):
  1. LN stats of x^T (bf16) via ones-matmul on PE; rsqrt on ACT.
     LN is algebraically folded into the projection epilogues:
       qkv^T[o,s] = r[s]*Y[o,s] - (r[s]*mu[s])*C[o] + b2[o],
     with Y = (wqkv*ln_g)^T @ x^T, C[o] = sum_f Wg[f,o].
  2. QK projection -> qk^T [512, 2048]; V projection with swapped operands
     -> V in row layout [2048, 256] (+ a constant ones column per head for
     softmax denominators).
  3. Per head: scores^T blocks [128 k, q] for visible (causal) q-range only,
     exp on ACT (scale=1/8 folded in, no max-subtraction: |scores| is small
     and masked lanes underflow to exact 0 like the reference's -1e4), a
     128x128 triangular mask on the diagonal block, then ctx^T = V_aug^T @
     expS accumulated in PSUM ([65, 512]: row 64 = softmax denominator).
  4. Normalize ctx rows by 1/denominator (broadcast via DRAM bounce).
  5. AllToAll within the 4-core batch group -> full-feature ctx^T for this
     core's seq chunk; output projection + bias + residual; host reassembles.
"""

import sys

for _p in ("/opt/trn_rl_repo",):
    if _p not in sys.path:
        sys.path.insert(0, _p)

import math

import ml_dtypes
import numpy as np

import concourse.bass as bass
import concourse.mybir as mybir
import concourse.tile as tile
from concourse import bacc
from concourse.bass_utils import run_bass_kernel_spmd

F32 = mybir.dt.float32
BF16 = mybir.dt.bfloat16
AF = mybir.ActivationFunctionType
ALU = mybir.AluOpType

N_CORES = 8
B, S, H, D = 2, 2048, 16, 64
DIM = H * D              # 1024
HL = 4                   # heads per core
DL = HL * D              # 256 local head features
SC = S // 4              # 512 seq chunk per core
WC = S // N_CORES        # 256: output window per core (per batch)
EPS = 1e-6
KT = 128                 # k-tile (partition) width
NT = 512                 # matmul free-dim tile
FT = DIM // KT           # 8 feature tiles
ST = S // KT             # 16 seq tiles of 128
QT = S // NT             # 4 q-tiles of 512

_CACHE = {}


def _build():
    nc = bacc.Bacc("TRN2", target_bir_lowering=False, debug=False,
                   num_devices=N_CORES)

    # ---- I/O ----
    xbf_d = nc.dram_tensor("xbf", [DIM, S], BF16, kind="ExternalInput")
    xres_d = nc.dram_tensor("xres", [DIM, SC], F32, kind="ExternalInput")
    wqk_d = nc.dram_tensor("wqk", [DIM, 2 * DL], BF16, kind="ExternalInput")
    wv_d = nc.dram_tensor("wv", [DIM, DL], BF16, kind="ExternalInput")
    wo_d = nc.dram_tensor("wo", [DIM, DIM], BF16, kind="ExternalInput")
    cqk_d = nc.dram_tensor("cqk", [128, 4], F32, kind="ExternalInput")
    bqk_d = nc.dram_tensor("bqk", [128, 4], F32, kind="ExternalInput")
    cvb_d = nc.dram_tensor("cvb", [128, DL], F32, kind="ExternalInput")
    bvb_d = nc.dram_tensor("bvb", [128, DL], F32, kind="ExternalInput")
    tri_d = nc.dram_tensor("tri", [128, 128], BF16, kind="ExternalInput")
    bo_d = nc.dram_tensor("bo_col", [128, FT], F32, kind="ExternalInput")
    y_d = nc.dram_tensor("y", [DIM, SC], F32, kind="ExternalOutput")

    # ---- DRAM scratch ----
    stats_dram = nc.dram_tensor("stats_dram", [2, S], F32)
    r_dram = nc.dram_tensor("r_dram", [S], F32)
    m2_dram = nc.dram_tensor("m2_dram", [S], F32)
    rec_dram = nc.dram_tensor("rec_dram", [HL * QT, NT], F32)
    rec2_dram = nc.dram_tensor("rec2_dram", [HL * QT, NT], F32)
    a2a_in = nc.dram_tensor("a2a_in", [N_CORES, DL + 8, WC], BF16)
    a2a_out = nc.dram_tensor("a2a_out", [N_CORES, DL + 8, WC], BF16)
    rec3_dram = nc.dram_tensor("rec3_dram", [N_CORES, 4, WC], F32)

    with tile.TileContext(nc) as tc:
        import contextlib
        with contextlib.ExitStack() as ctx:
            _build_body(ctx, tc, nc, locals())
    nc.compile()
    return nc


def _build_body(ctx, tc, nc, t):
    xbf_d, xres_d, wqk_d, wv_d, wo_d = (t["xbf_d"], t["xres_d"], t["wqk_d"],
                                        t["wv_d"], t["wo_d"])
    cqk_d, bqk_d, cvb_d, bvb_d, tri_d, bo_d, y_d = (
        t["cqk_d"], t["bqk_d"], t["cvb_d"], t["bvb_d"], t["tri_d"],
        t["bo_d"], t["y_d"])
    stats_dram, r_dram, m2_dram, rec_dram, a2a_in, a2a_out = (
        t["stats_dram"], t["r_dram"], t["m2_dram"], t["rec_dram"],
        t["a2a_in"], t["a2a_out"])
    rec2_dram = t["rec2_dram"]
    rec3_dram = t["rec3_dram"]

    P = 128
    sing = ctx.enter_context(tc.tile_pool(name="sing", bufs=1))
    # persistent SBUF tiles
    xbf = [sing.tile([P, S], BF16, tag=f"xbf{i}", name=f"xbf{i}") for i in range(FT)]
    xres = [sing.tile([P, SC], F32, tag=f"xres{i}", name=f"xres{i}") for i in range(FT)]
    wqk = [sing.tile([P, 2 * DL], BF16, tag=f"wqk{i}", name=f"wqk{i}") for i in range(FT)]
    wv = [sing.tile([P, DL], BF16, tag=f"wv{i}", name=f"wv{i}") for i in range(FT)]
    wo = [sing.tile([P, DIM], BF16, tag=f"wo{i}", name=f"wo{i}") for i in range(FT)]
    qkT = [sing.tile([P, S], BF16, tag=f"qkT{i}", name=f"qkT{i}") for i in range(4)]
    vaug = [sing.tile([P, HL * (D + 1)], BF16, tag=f"vaug{i}", name=f"vaug{i}")
            for i in range(ST)]
    ctxT = [sing.tile([P, S], BF16, tag=f"ctxT{i}", name=f"ctxT{i}") for i in range(2)]
    ctx_all = [sing.tile([P, WC], BF16, tag=f"call{i}", name=f"call{i}")
               for i in range(2 * FT)]
    rB = [sing.tile([P, NT], F32, tag=f"rB{i}", name=f"rB{i}") for i in range(QT)]
    m2B = [sing.tile([P, NT], F32, tag=f"m2B{i}", name=f"m2B{i}") for i in range(QT)]
    cqk_c = sing.tile([P, 4], F32, tag="cqk")
    bqk_c = sing.tile([P, 4], F32, tag="bqk")
    cvb = sing.tile([P, DL], F32, tag="cvb")
    bvb = sing.tile([P, DL], F32, tag="bvb")
    tri = sing.tile([P, P], BF16, tag="tri")
    bo_c = sing.tile([P, FT], F32, tag="bo")
    ones = sing.tile([P, 1], BF16, tag="ones")

    rcP = sing.tile([P, ST], F32, tag="rcP")
    m2P = sing.tile([P, ST], F32, tag="m2P")

    # input DMAs — xbf first (stats critical path), weights next, rest last
    for i in range(FT):
        nc.sync.dma_start(out=xbf[i], in_=xbf_d[i * P:(i + 1) * P, :])
    for i in range(FT):
        nc.sync.dma_start(out=wqk[i], in_=wqk_d[i * P:(i + 1) * P, :])
    for i in range(FT):
        nc.sync.dma_start(out=wv[i], in_=wv_d[i * P:(i + 1) * P, :])
    nc.sync.dma_start(out=cqk_c, in_=cqk_d[:])
    nc.sync.dma_start(out=bqk_c, in_=bqk_d[:])
    nc.sync.dma_start(out=cvb, in_=cvb_d[:])
    nc.sync.dma_start(out=bvb, in_=bvb_d[:])
    nc.sync.dma_start(out=tri, in_=tri_d[:])
    nc.sync.dma_start(out=bo_c, in_=bo_d[:])
    for i in range(FT):
        nc.sync.dma_start(out=wo[i], in_=wo_d[i * P:(i + 1) * P, :])
        nc.sync.dma_start(out=xres[i], in_=xres_d[i * P:(i + 1) * P, :])
    nc.vector.memset(ones, 1.0)

    # ---- 1. LN stats: column sums of x and x^2 via ones-matmul ----
    with tc.tile_pool(name="ps_st", bufs=4, space="PSUM") as ps_st, \
         tc.tile_pool(name="sqp", bufs=2) as sqp:
        stats_sa = sqp.tile([1, S], F32, tag="stats_sa", bufs=1)
        stats_sq = sqp.tile([1, S], F32, tag="stats_sq", bufs=1)
        sps = [ps_st.tile([1, NT], F32, tag="sum", name=f"sum{nt}")
               for nt in range(QT)]
        qps = [ps_st.tile([1, NT], F32, tag="sq", name=f"sqp{nt}")
               for nt in range(QT)]
        for k in range(FT):
            sq = sqp.tile([P, S], BF16, tag="sq", name="sq")
            nc.vector.tensor_mul(sq, xbf[k], xbf[k])
            for nt in range(QT):
                sl = slice(nt * NT, (nt + 1) * NT)
                nc.tensor.matmul(sps[nt], ones, xbf[k][:, sl],
                                 start=(k == 0), stop=(k == FT - 1))
                nc.tensor.matmul(qps[nt], ones, sq[:, sl],
                                 start=(k == 0), stop=(k == FT - 1))
        for nt in range(QT):
            sl = slice(nt * NT, (nt + 1) * NT)
            nc.vector.tensor_copy(stats_sa[:, sl], sps[nt])
            nc.vector.tensor_copy(stats_sq[:, sl], qps[nt])
    nc.sync.dma_start(out=stats_dram[0:1], in_=stats_sa[:])
    nc.sync.dma_start(out=stats_dram[1:2], in_=stats_sq[:])
    # fast path: contiguous [16,128] reads, math at 16 partitions, then
    # flatten (SBUF->SBUF DMA) for the PE row-broadcasts and PE-transpose
    # for the [128,16] column layout used by the V epilogue
    idn = sing.tile([P, P], F32, tag="idn")
    from concourse.masks import make_identity
    make_identity(nc, idn)
    sPT = sing.tile([16, P], F32, tag="sPT")
    qPT = sing.tile([16, P], F32, tag="qPT")
    nc.sync.dma_start(out=sPT, in_=stats_dram[0].rearrange("(j p) -> j p", j=16))
    nc.sync.dma_start(out=qPT, in_=stats_dram[1].rearrange("(j p) -> j p", j=16))
    muT = sing.tile([16, P], F32, tag="muT")
    nc.vector.tensor_scalar(muT, sPT, 1.0 / DIM, None, op0=ALU.mult)
    nc.vector.tensor_scalar(qPT, qPT, 1.0 / DIM, None, op0=ALU.mult)
    t0 = sing.tile([16, P], F32, tag="t0")
    nc.vector.tensor_mul(t0, muT, muT)
    nc.vector.tensor_sub(t0, qPT, t0)
    nc.vector.tensor_scalar(t0, t0, EPS, None, op0=ALU.add)
    # rsqrt via fast-inverse-square-root seed + 3 Newton steps (no tables)
    I32 = mybir.dt.int32
    rT = sing.tile([16, P], F32, tag="rT")
    t1s = sing.tile([16, P], F32, tag="t1s")
    nc.vector.tensor_scalar(rT[:].bitcast(I32), t0[:].bitcast(I32), 1, None,
                            op0=ALU.logical_shift_right)
    nc.vector.tensor_scalar(rT[:].bitcast(I32), rT[:].bitcast(I32), -1, None,
                            op0=ALU.bitwise_xor)
    nc.vector.tensor_scalar(rT[:].bitcast(I32), rT[:].bitcast(I32),
                            0x5F3759E0, None, op0=ALU.add)
    for _ in range(3):
        nc.vector.tensor_mul(t1s, rT, rT)
        nc.vector.tensor_mul(t1s, t1s, t0)
        nc.vector.tensor_scalar(t1s, t1s, -0.5, 1.5, op0=ALU.mult,
                                op1=ALU.add)
        nc.vector.tensor_mul(rT, rT, t1s)
    m2T = sing.tile([16, P], F32, tag="m2T")
    nc.vector.tensor_mul(m2T, muT, rT)
    with tc.tile_pool(name="ps_bc", bufs=2, space="PSUM") as ps_bc, \
         tc.tile_pool(name="bcs", bufs=1) as bcs:
        r_row = bcs.tile([1, S], F32, tag="r_row")
        m2_row = bcs.tile([1, S], F32, tag="m2_row")
        nc.sync.dma_start(out=r_row, in_=rT[:])
        nc.sync.dma_start(out=m2_row, in_=m2T[:])
        ones_row = bcs.tile([1, P], F32, tag="ones_row")
        nc.vector.memset(ones_row, 1.0)
        for nt in range(QT):
            sl = slice(nt * NT, (nt + 1) * NT)
            bp = ps_bc.tile([P, NT], F32, tag="bc", name="bc")
            nc.tensor.matmul(bp, ones_row, r_row[:, sl], start=True, stop=True)
            nc.scalar.copy(rB[nt], bp)
            bp2 = ps_bc.tile([P, NT], F32, tag="bc", name="bc")
            nc.tensor.matmul(bp2, ones_row, m2_row[:, sl], start=True,
                             stop=True)
            nc.scalar.copy(m2B[nt], bp2)
        # column layout for the V epilogue via PE transpose
        tp = ps_bc.tile([P, 16], F32, tag="tp", name="tp")
        nc.tensor.transpose(tp, rT[:], idn[0:16, 0:16])
        nc.vector.tensor_copy(rcP, tp)
        tp2 = ps_bc.tile([P, 16], F32, tag="tp", name="tp")
        nc.tensor.transpose(tp2, m2T[:], idn[0:16, 0:16])
        nc.vector.tensor_copy(m2P, tp2)

    # ---- 2. QK projection (V is woven into the attention loop) ----
    with tc.tile_pool(name="ps_qk", bufs=5, space="PSUM") as ps_qk, \
         tc.tile_pool(name="tmp", bufs=3) as tmp:
        for mt in range(4):          # qkT M-tiles (Q01 Q23 K01 K23)
            for nt in range(QT):
                sl = slice(nt * NT, (nt + 1) * NT)
                ps = ps_qk.tile([P, NT], F32, tag="qk", name="qk")
                for k in range(FT):
                    nc.tensor.matmul(
                        ps, wqk[k][:, mt * P:(mt + 1) * P], xbf[k][:, sl],
                        start=(k == 0), stop=(k == FT - 1))
                t1 = tmp.tile([P, NT], F32, tag="t1")
                nc.vector.tensor_mul(t1, ps, rB[nt])
                t2 = tmp.tile([P, NT], F32, tag="t2")
                nc.vector.tensor_scalar(
                    t2, m2B[nt], cqk_c[:, mt:mt + 1], bqk_c[:, mt:mt + 1],
                    op0=ALU.mult, op1=ALU.subtract)
                nc.vector.tensor_sub(qkT[mt][:, sl], t1, t2)

    # ---- 3. attention: q-outer, 4 heads interleaved, V proj woven in ----
    with tc.tile_pool(name="ps_sc", bufs=3, space="PSUM") as ps_sc, \
         tc.tile_pool(name="ps_cx", bufs=1, space="PSUM") as ps_cx, \
         tc.tile_pool(name="ps_v", bufs=1, space="PSUM") as ps_v, \
         tc.tile_pool(name="esp", bufs=8) as esp, \
         tc.tile_pool(name="vtmp", bufs=2) as vtmp, \
         tc.tile_pool(name="recp", bufs=2) as recp:
        v_done = set()

        def weave_v(st):
            if st in v_done:
                return
            v_done.add(st)
            ps = ps_v.tile([P, DL], F32, tag="v", name="v")
            for k in range(FT):
                nc.tensor.matmul(
                    ps, xbf[k][:, st * P:(st + 1) * P], wv[k],
                    start=(k == 0), stop=(k == FT - 1))
            u1 = vtmp.tile([P, DL], F32, tag="u1", name="u1")
            nc.vector.tensor_scalar(u1, ps, rcP[:, st:st + 1], None,
                                    op0=ALU.mult)
            u2 = vtmp.tile([P, DL], F32, tag="u2", name="u2")
            nc.vector.tensor_scalar(u2, cvb, m2P[:, st:st + 1], None,
                                    op0=ALU.mult)
            nc.vector.tensor_sub(u1, u1, u2)
            nc.vector.tensor_add(
                vaug[st][:].rearrange("p (h e) -> p h e", h=HL)[:, :, 0:D],
                u1.rearrange("p (h d) -> p h d", h=HL),
                bvb.rearrange("p (h d) -> p h d", h=HL))
            nc.vector.memset(
                vaug[st][:].rearrange("p (h e) -> p h e", h=HL)[:, :,
                                                                D:D + 1],
                1.0)

        for qt in range(QT):
            q0 = qt * NT
            cxs = [ps_cx.tile([D + 1, NT], F32, tag=f"cx{hl}",
                              name=f"cx{hl}") for hl in range(HL)]
            for kt in range(4 * qt + 4):
                k0 = kt * KT
                weave_v(kt)
                dlt = k0 - q0          # >0 only on diagonal k-tiles
                ess = []
                for hl in range(HL):
                    hg, u = hl // 2, hl % 2
                    hp = slice(64 * u, 64 * u + 64)
                    st = ps_sc.tile([P, NT], F32, tag="sc", name="sc")
                    es = esp.tile([P, NT], BF16, tag="es", name="es")
                    if dlt > 0:
                        nc.vector.memset(es[:, 0:dlt], 0.0)
                        nc.tensor.matmul(st[:, dlt:], qkT[2 + hg][hp,
                                                                  k0:k0 + KT],
                                         qkT[hg][hp, q0 + dlt:q0 + NT],
                                         start=True, stop=True)
                        nc.scalar.activation(es[:, dlt:], st[:, dlt:],
                                             AF.Exp, scale=1.0 / math.sqrt(D))
                    else:
                        nc.tensor.matmul(st, qkT[2 + hg][hp, k0:k0 + KT],
                                         qkT[hg][hp, q0:q0 + NT],
                                         start=True, stop=True)
                        nc.scalar.activation(es, st, AF.Exp,
                                             scale=1.0 / math.sqrt(D))
                    ess.append(es)
                if dlt >= 0 and kt >= 4 * qt:   # diagonal triangle
                    for hl in range(HL):
                        nc.vector.tensor_mul(ess[hl][:, dlt:dlt + KT],
                                             ess[hl][:, dlt:dlt + KT], tri)
                for hl in range(HL):
                    nc.tensor.matmul(
                        cxs[hl], vaug[kt][:, hl * (D + 1):(hl + 1) * (D + 1)],
                        ess[hl][:, :],
                        start=(kt == 0), stop=(kt == 4 * qt + 3))
            for hl in range(HL):
                row = hl * QT + qt
                rec = recp.tile([1, NT], F32, tag="rec", name="rec")
                nc.vector.tensor_copy(rec, cxs[hl][D:D + 1, :])
                nc.sync.dma_start(out=rec_dram[row:row + 1], in_=rec)
                nc.vector.tensor_copy(
                    ctxT[hl // 2][64 * (hl % 2):64 * (hl % 2) + 64,
                                  q0:q0 + NT],
                    cxs[hl][0:D, :])
            # ship this qt's two windows (unnormalized ctx + raw sums)
            recv = rec_dram[:].rearrange("(h q) c -> h q c", q=QT)
            for j in (2 * qt, 2 * qt + 1):
                sl = slice(j * WC, (j + 1) * WC)
                nc.sync.dma_start(out=a2a_in[j, 0:P, :], in_=ctxT[0][:, sl])
                nc.sync.dma_start(out=a2a_in[j, P:2 * P, :],
                                  in_=ctxT[1][:, sl])
                off = (j % 2) * WC
                nc.sync.dma_start(
                    out=a2a_in[j, DL:DL + 8, :].rearrange(
                        "a b -> (a b)").bitcast(F32).rearrange(
                        "(a b) -> a b", a=HL),
                    in_=recv[:, qt, off:off + WC])

    # ---- 4. rank-symmetric AllToAll (all 8 received blocks useful) ----
    nc.gpsimd.collective_compute(
        "AllToAll", ALU.bypass,
        replica_groups=[list(range(N_CORES))],
        ins=[a2a_in[:].opt()], outs=[a2a_out[:].opt()],
        unique_tensors="Yes")

    # ---- 5. receiver: 1/sums (Newton), normalize, project, residual ----
    with tc.tile_pool(name="ps_o", bufs=2, space="PSUM") as ps_o, \
         tc.tile_pool(name="ps_rb", bufs=2, space="PSUM") as ps_rb, \
         tc.tile_pool(name="yp", bufs=2) as yp, \
         tc.tile_pool(name="rbp", bufs=3) as rbp:
        # NR reciprocal of all shipped sums: bytes land as [64, 128] f32
        s4 = rbp.tile([64, 128], F32, tag="s4", name="s4", bufs=1)
        for g in range(N_CORES):
            nc.sync.dma_start(out=s4[:].bitcast(BF16)[8 * g:8 * g + 8, :],
                              in_=a2a_out[g, DL:DL + 8, :])
        y0 = rbp.tile([64, 128], F32, tag="y0", name="y0", bufs=1)
        a4 = rbp.tile([64, 128], F32, tag="a4", name="a4", bufs=1)
        I32 = mybir.dt.int32
        nc.vector.tensor_scalar(y0[:].bitcast(I32), s4[:].bitcast(I32),
                                -1, None, op0=ALU.bitwise_xor)
        nc.vector.tensor_scalar(y0[:].bitcast(I32), y0[:].bitcast(I32),
                                0x7EF311C4, None, op0=ALU.add)
        for _ in range(3):
            nc.vector.tensor_mul(a4, s4, y0)
            nc.vector.tensor_scalar(a4, a4, -1.0, 2.0,
                                    op0=ALU.mult, op1=ALU.add)
            nc.vector.tensor_mul(y0, y0, a4)
        # flatten reciprocals to one row (partition-major byte order)
        yrow = rbp.tile([1, 64 * 128], F32, tag="yrow", name="yrow", bufs=1)
        nc.sync.dma_start(out=yrow, in_=y0)
        # pull ctx blocks, normalize each feature tile, then project
        flat = a2a_out[:].rearrange("g p q -> (g p) q")
        ones_rb = rbp.tile([1, P], F32, tag="ones_rb", name="ones_rb")
        nc.vector.memset(ones_rb, 1.0)
        for b2 in range(2):
            for i in range(FT):
                src = i // 2 + 4 * b2
                nc.sync.dma_start(
                    out=ctx_all[b2 * FT + i],
                    in_=flat[src * (DL + 8) + P * (i % 2):
                             src * (DL + 8) + P * (i % 2) + P, :])
                rb = rbp.tile([P, WC], F32, tag="rb", name="rb", bufs=2)
                bp = ps_rb.tile([P, WC], F32, tag="rbp", name="rbp")
                for u in range(2):
                    lh = 2 * (i % 2) + u
                    off = src * 1024 + lh * WC
                    nc.tensor.matmul(bp[64 * u:64 * u + 64, :],
                                     ones_rb[:, 0:64],
                                     yrow[:, off:off + WC],
                                     start=True, stop=True)
                nc.scalar.copy(rb, bp)
                nc.vector.tensor_mul(ctx_all[b2 * FT + i],
                                     ctx_all[b2 * FT + i], rb)
            for mt in range(FT):
                ps = ps_o.tile([P, WC], F32, tag="o", name="o")
                for k in range(FT):
                    nc.tensor.matmul(ps, wo[k][:, mt * P:(mt + 1) * P],
                                     ctx_all[b2 * FT + k],
                                     start=(k == 0), stop=(k == FT - 1))
                ysb = yp.tile([P, WC], F32, tag="y", name="y")
                nc.vector.tensor_add(ysb, ps,
                                     xres[mt][:, b2 * WC:(b2 + 1) * WC])
                nc.scalar.activation(ysb, ysb, AF.Identity,
                                     bias=bo_c[:, mt:mt + 1])
                nc.sync.dma_start(
                    out=y_d[mt * P:(mt + 1) * P, b2 * WC:(b2 + 1) * WC],
                    in_=ysb)


def _prep_inputs(x, ln_g, ln_b, wqkv, bqkv, wo, bo):
    """Host-side sharding / folding. Returns per-core input dicts."""
    f32 = np.float32
    bf16 = ml_dtypes.bfloat16
    x = np.asarray(x, f32)
    wg = (np.asarray(wqkv, f32) * np.asarray(ln_g, f32)[:, None])
    tri = (np.arange(128)[None, :] >= np.arange(128)[:, None]).astype(bf16)
    wo_bf = np.asarray(wo, f32).astype(bf16)
    bo_col = np.ascontiguousarray(np.asarray(bo, f32).reshape(FT, 128).T)
    lnb = np.asarray(ln_b, f32)
    bq = np.asarray(bqkv, f32)

    xT = [np.ascontiguousarray(x[b].T) for b in range(B)]
    xbf = [t.astype(bf16) for t in xT]

    maps = []
    for c in range(N_CORES):
        b, s = divmod(c, 4)
        qs = slice(DL * s, DL * s + DL)
        ks = slice(DIM + DL * s, DIM + DL * s + DL)
        vs = slice(2 * DIM + DL * s, 2 * DIM + DL * s + DL)
        wqk_l = np.concatenate([wg[:, qs], wg[:, ks]], axis=1).astype(bf16)
        wv_l = wg[:, vs].astype(bf16)
        wqk_f = wqk_l.astype(f32)
        wv_f = wv_l.astype(f32)
        cqk = wqk_f.sum(0)                       # [512]
        b2qk = np.concatenate([bq[qs], bq[ks]]) + wqk_f.T @ lnb
        cv = wv_f.sum(0)                         # [256]
        b2v = bq[vs] + wv_f.T @ lnb
        win = slice(WC * c, WC * c + WC)
        maps.append({
            "xbf": xbf[b],
            "xres": np.ascontiguousarray(
                np.concatenate([xT[0][:, win], xT[1][:, win]], axis=1)),
            "wqk": wqk_l,
            "wv": wv_l,
            "wo": wo_bf,
            "cqk": np.ascontiguousarray(cqk.reshape(4, 128).T),
            "bqk": np.ascontiguousarray(b2qk.reshape(4, 128).T),
            "cvb": np.ascontiguousarray(np.broadcast_to(cv, (128, DL))).astype(f32),
            "bvb": np.ascontiguousarray(np.broadcast_to(b2v, (128, DL))).astype(f32),
            "tri": tri,
            "bo_col": bo_col,
        })
    return maps


def kernel(**inputs):
    if "nc" not in _CACHE:
        _CACHE["nc"] = _build()
    nc = _CACHE["nc"]
    maps = _prep_inputs(**inputs)
    res = run_bass_kernel_spmd(nc, maps, list(range(N_CORES)))
    out = np.empty((B, S, DIM), np.float32)
    for c in range(N_CORES):
        y = res.results[c]["y"]            # [DIM, 2*WC]
        out[0, WC * c:WC * c + WC, :] = y[:, :WC].T
        out[1, WC * c:WC * c + WC, :] = y[:, WC:].T
    return out

